# revision 18
# baseline (speedup 1.0000x reference)
"""Trainium2 Bass kernel for nn_AttentionLayer_77558519431766.

Math: the reference computes softmax over a size-1 axis, which is
identically 1.0, so the attention MLP is dead code and

    out[b, e] = sum_{i<j} x[b,i,e] * x[b,j,e]
              = 0.5 * ((sum_f x[b,f,e])^2 - sum_f x[b,f,e]^2)

Implementation (per 128-sample chunk, layout [128b, f*64+e]):
  1. ACT casts x to bf16.
  2. PE transposes each [128b, 128(f2,e)] block into PSUM, so pairs of
     f-rows land on partitions.
  3. DVE copies the transposed blocks back to SBUF (for s), ACT squares
     them into SBUF (for q).
  4. PE runs two matmul accumulation chains against a stacked-identity
     mask [128,64] (row (f2,e) is one-hot at e), yielding
     s = sum_f x and q = sum_f x^2 as [128b, 64e] in PSUM.
  5. res = 0.5*s^2 - 0.5*q, DMA out.

Sharding: pure data parallelism, batch 2048 -> 8 shards of 256.
"""

import numpy as np

try:
    import concourse.bass as bass  # noqa: F401
except ImportError:  # pragma: no cover
    import sys

    sys.path.insert(0, "/opt/trn_rl_repo")

_B, _F, _E = 2048, 50, 64
_NCORES = 8
_BS = _B // _NCORES  # 256 rows per core
_ROW = _F * _E  # 3200 floats per row
_P = 128  # SBUF partitions
_NBLK = _ROW // _P  # 25 transpose blocks per chunk


def _make_tc_class():
    """TileContext with a slim kernel tail.

    Stock TileContext ends with drain -> full all-engine barrier ->
    semaphore clear -> second full barrier (~6-8us of EVSEM butterfly).
    The Bass preamble already dma_reset+sem_clears the entire kernel
    semaphore range at the start of every execution, so the tail clear
    and second barrier are redundant for a single-TileContext kernel.
    Keep the global-clock drain (output DMA completion) plus one cheap
    sequencer-level barrier.
    """
    from concourse.tile import TileContext
    from concourse.vector_clock import ScopedClock

    class SlimTailTileContext(TileContext):
        def _drain_and_barrier(self, tick_clock, wait_clock):
            drain_inst = self.nc.sync.drain()
            wait_clock.add_sem_waits(
                drain_inst.ins, ScopedClock({None: tick_clock.global_clock})
            )
            self.nc.all_engine_barrier(sem_only=True)
            popped = self.nc._tile_sem_poison_stack.pop()
            assert popped is self._sem_poison

    return SlimTailTileContext


def _build():
    import concourse.bacc as bacc
    import concourse.mybir as mybir

    TileContext = _make_tc_class()

    f32 = mybir.dt.float32
    bf16 = mybir.dt.bfloat16
    SQ = mybir.ActivationFunctionType.Square
    CP = mybir.ActivationFunctionType.Copy
    ALU = mybir.AluOpType
    HALF_SQRT = float(np.float32(np.sqrt(0.5)))

    nc = bacc.Bacc()
    x = nc.declare_dram_parameter("inputs", [_BS, _ROW], f32, isOutput=False)
    ident_d = nc.declare_dram_parameter("ident", [_P, _P], bf16, isOutput=False)
    mask_d = nc.declare_dram_parameter("mask", [_P, _E], bf16, isOutput=False)
    out = nc.declare_dram_parameter("out", [_BS, _E], f32, isOutput=True)

    n_chunks = _BS // _P  # 2
    halves = [(0, _NBLK)]  # full-width DMA keeps 12.8KB/row packets (345 GB/s)
    # transpose blocks per PSUM tile; a [128, 13*128] bf16 tile spans 2 banks
    # but every transpose output ([128,128] bf16 = 256B/partition) stays
    # inside one bank since 256 divides 2048.
    groups = [13, 12]

    with TileContext(nc) as tc:
        with (
            tc.tile_pool(name="consts", bufs=1) as cpool,
            tc.tile_pool(name="x", bufs=4) as xpool,
            tc.tile_pool(name="xb", bufs=4) as xbpool,
            tc.tile_pool(name="xT", bufs=4) as xtpool,
            tc.tile_pool(name="xsq", bufs=4) as sqpool,
            tc.tile_pool(name="pt", bufs=2, space="PSUM") as ptpool,
            tc.tile_pool(name="acc", bufs=2, space="PSUM") as accpool,
            tc.tile_pool(name="small", bufs=2) as spool,
        ):
            # Warm op: forces the ACT function-table load off the critical
            # path (it otherwise lands right before the first Square, after
            # a cross-engine wait).
            warm = spool.tile([_P, 1], f32, tag="warm")
            nc.gpsimd.memset(warm[:], 0.0)
            nc.scalar.activation(warm[:], warm[:], SQ)

            # Consts go on the Scalar HWDGE queue so they don't queue
            # behind the big input DMAs on the Sync queue.
            ident = cpool.tile([_P, _P], bf16, tag="ident")
            mask = cpool.tile([_P, _E], bf16, tag="mask")
            nc.scalar.dma_start(out=ident[:], in_=ident_d[:])
            nc.scalar.dma_start(out=mask[:], in_=mask_d[:])

            for c in range(n_chunks):
                rows = slice(c * _P, (c + 1) * _P)
                # separate banks: a start=True matmul clears its whole bank,
                # so the two accumulation chains must not share one
                s_t = accpool.tile([_P, _E], f32, tag="s")
                q_t = accpool.tile([_P, _E], f32, tag="q")
                s_ps = s_t[:]
                q_ps = q_t[:]
                for blk0, nblk in halves:
                    cols = slice(blk0 * _P, (blk0 + nblk) * _P)
                    n = nblk * _P
                    xt = xpool.tile([_P, n], f32, tag="x")
                    nc.sync.dma_start(out=xt[:], in_=x[rows, cols])
                    xbt = xbpool.tile([_P, n], bf16, tag="xb")
                    nc.vector.tensor_copy(xbt[:], xt[:])

                    xT = xtpool.tile([_P, n], bf16, tag="xT")
                    xsq = sqpool.tile([_P, n], bf16, tag="xsq")
                    g0 = 0
                    for gn in groups:
                        pt = ptpool.tile([_P, groups[0] * _P], bf16, tag="pt")
                        for j in range(gn):
                            k = g0 + j
                            nc.tensor.transpose(
                                pt[:, j * _P : (j + 1) * _P],
                                xbt[:, k * _P : (k + 1) * _P],
                                ident[:],
                            )
                        gcols = slice(g0 * _P, (g0 + gn) * _P)
                        nc.vector.tensor_copy(xT[:, gcols], pt[:, : gn * _P])
                        nc.scalar.activation(xsq[:, gcols], pt[:, : gn * _P], SQ)
                        g0 += gn
                    for k in range(nblk):
                        kk = blk0 + k
                        bcols = slice(k * _P, (k + 1) * _P)
                        nc.tensor.matmul(
                            s_ps,
                            xT[:, bcols],
                            mask[:],
                            start=(kk == 0),
                            stop=(kk == _NBLK - 1),
                        )
                        nc.tensor.matmul(
                            q_ps,
                            xsq[:, bcols],
                            mask[:],
                            start=(kk == 0),
                            stop=(kk == _NBLK - 1),
                        )

                # res = 0.5*s^2 - 0.5*q
                s2h = spool.tile([_P, _E], f32, tag="s2h")
                res = spool.tile([_P, _E], f32, tag="res")
                nc.scalar.activation(s2h[:], s_ps, SQ, scale=HALF_SQRT)
                nc.vector.scalar_tensor_tensor(
                    res[:], q_ps, -0.5, s2h[:], op0=ALU.mult, op1=ALU.add
                )
                nc.sync.dma_start(out=out[rows, :], in_=res[:])
    nc.compile()
    return nc


def _consts():
    import ml_dtypes

    ident = np.eye(_P, dtype=ml_dtypes.bfloat16)
    mask = np.zeros((_P, _E), dtype=ml_dtypes.bfloat16)
    for j in range(_P):
        mask[j, j % _E] = 1
    return ident, mask


def _run(in_maps, **kwargs):
    from concourse.bass_utils import run_bass_kernel_spmd

    nc = _build()
    return run_bass_kernel_spmd(nc, in_maps, core_ids=list(range(_NCORES)), **kwargs)


def _shard(inputs: np.ndarray):
    x = np.ascontiguousarray(
        np.asarray(inputs, dtype=np.float32).reshape(_B, _ROW)
    )
    ident, mask = _consts()
    return [
        {
            "inputs": np.ascontiguousarray(x[i * _BS : (i + 1) * _BS]),
            "ident": ident,
            "mask": mask,
        }
        for i in range(_NCORES)
    ]


def kernel(
    inputs: np.ndarray,
    weight_attention: np.ndarray = None,
    weight_projection: np.ndarray = None,
    weight_bias: np.ndarray = None,
) -> np.ndarray:
    # weights are dead code (softmax over a size-1 axis == 1.0)
    res = _run(_shard(inputs))
    return np.concatenate([r["out"] for r in res.results], axis=0)


# revision 20
# speedup vs baseline: 1.0845x; 1.0845x over previous
"""Trainium2 Bass kernel for nn_AttentionLayer_77558519431766.

Math: the reference computes softmax over a size-1 axis, which is
identically 1.0, so the attention MLP is dead code and

    out[b, e] = sum_{i<j} x[b,i,e] * x[b,j,e]
              = 0.5 * ((sum_f x[b,f,e])^2 - sum_f x[b,f,e]^2)

Implementation (per 128-sample chunk, layout [128b, f*64+e]):
  1. ACT casts x to bf16.
  2. PE transposes each [128b, 128(f2,e)] block into PSUM, so pairs of
     f-rows land on partitions.
  3. DVE copies the transposed blocks back to SBUF (for s), ACT squares
     them into SBUF (for q).
  4. PE runs two matmul accumulation chains against a stacked-identity
     mask [128,64] (row (f2,e) is one-hot at e), yielding
     s = sum_f x and q = sum_f x^2 as [128b, 64e] in PSUM.
  5. res = 0.5*s^2 - 0.5*q, DMA out.

Sharding: pure data parallelism, batch 2048 -> 8 shards of 256.
"""

import numpy as np

try:
    import concourse.bass as bass  # noqa: F401
except ImportError:  # pragma: no cover
    import sys

    sys.path.insert(0, "/opt/trn_rl_repo")

_B, _F, _E = 2048, 50, 64
_NCORES = 8
_BS = _B // _NCORES  # 256 rows per core
_ROW = _F * _E  # 3200 floats per row
_P = 128  # SBUF partitions
_NBLK = _ROW // _P  # 25 transpose blocks per chunk


def _make_tc_class():
    """TileContext with a slim kernel tail.

    Stock TileContext ends with drain -> full all-engine barrier ->
    semaphore clear -> second full barrier (~6-8us of EVSEM butterfly).
    The Bass preamble already dma_reset+sem_clears the entire kernel
    semaphore range at the start of every execution, so the tail clear
    and second barrier are redundant for a single-TileContext kernel.
    Keep the global-clock drain (output DMA completion) plus one cheap
    sequencer-level barrier.
    """
    from concourse.tile import TileContext
    from concourse.vector_clock import ScopedClock

    class SlimTailTileContext(TileContext):
        def _drain_and_barrier(self, tick_clock, wait_clock):
            drain_inst = self.nc.sync.drain()
            wait_clock.add_sem_waits(
                drain_inst.ins, ScopedClock({None: tick_clock.global_clock})
            )
            self.nc.all_engine_barrier(sem_only=True)
            popped = self.nc._tile_sem_poison_stack.pop()
            assert popped is self._sem_poison

    return SlimTailTileContext


def _build():
    import concourse.bacc as bacc
    import concourse.mybir as mybir

    TileContext = _make_tc_class()

    f32 = mybir.dt.float32
    bf16 = mybir.dt.bfloat16
    SQ = mybir.ActivationFunctionType.Square
    CP = mybir.ActivationFunctionType.Copy
    ALU = mybir.AluOpType
    HALF_SQRT = float(np.float32(np.sqrt(0.5)))

    nc = bacc.Bacc()
    x = nc.declare_dram_parameter("inputs", [_BS, _ROW], f32, isOutput=False)
    ident_d = nc.declare_dram_parameter("ident", [_P, _P], bf16, isOutput=False)
    mask_d = nc.declare_dram_parameter("mask", [_P, _E], bf16, isOutput=False)
    out = nc.declare_dram_parameter("out", [_BS, _E], f32, isOutput=True)

    n_chunks = _BS // _P  # 2
    halves = [(0, _NBLK)]  # full-width DMA keeps 12.8KB/row packets (345 GB/s)
    groups = [7, 6, 6, 6]  # transpose blocks per PSUM tile (<= 1 bank bf16)

    with TileContext(nc) as tc:
        with (
            tc.tile_pool(name="consts", bufs=1) as cpool,
            tc.tile_pool(name="x", bufs=4) as xpool,
            tc.tile_pool(name="xb", bufs=4) as xbpool,
            tc.tile_pool(name="xT", bufs=4) as xtpool,
            tc.tile_pool(name="xsq", bufs=4) as sqpool,
            tc.tile_pool(name="pt", bufs=3, space="PSUM") as ptpool,
            tc.tile_pool(name="acc", bufs=2, space="PSUM") as accpool,
            tc.tile_pool(name="small", bufs=2) as spool,
        ):
            # Warm op: forces the ACT function-table load off the critical
            # path (it otherwise lands right before the first Square, after
            # a cross-engine wait).
            warm = spool.tile([_P, 1], f32, tag="warm")
            nc.gpsimd.memset(warm[:], 0.0)
            nc.scalar.activation(warm[:], warm[:], SQ)

            # Consts go on the Scalar HWDGE queue so they don't queue
            # behind the big input DMAs on the Sync queue.
            ident = cpool.tile([_P, _P], bf16, tag="ident")
            mask = cpool.tile([_P, _E], bf16, tag="mask")
            nc.scalar.dma_start(out=ident[:], in_=ident_d[:])
            nc.scalar.dma_start(out=mask[:], in_=mask_d[:])

            for c in range(n_chunks):
                rows = slice(c * _P, (c + 1) * _P)
                # separate banks: a start=True matmul clears its whole bank,
                # so the two accumulation chains must not share one
                s_t = accpool.tile([_P, _E], f32, tag="s")
                q_t = accpool.tile([_P, _E], f32, tag="q")
                s_ps = s_t[:]
                q_ps = q_t[:]
                for blk0, nblk in halves:
                    cols = slice(blk0 * _P, (blk0 + nblk) * _P)
                    n = nblk * _P
                    xt = xpool.tile([_P, n], f32, tag="x")
                    nc.sync.dma_start(out=xt[:], in_=x[rows, cols])
                    xbt = xbpool.tile([_P, n], bf16, tag="xb")
                    nc.vector.tensor_copy(xbt[:], xt[:])

                    xT = xtpool.tile([_P, n], bf16, tag="xT")
                    xsq = sqpool.tile([_P, n], bf16, tag="xsq")
                    g0 = 0
                    for gn in groups:
                        pt = ptpool.tile([_P, groups[0] * _P], bf16, tag="pt")
                        for j in range(gn):
                            k = g0 + j
                            nc.tensor.transpose(
                                pt[:, j * _P : (j + 1) * _P],
                                xbt[:, k * _P : (k + 1) * _P],
                                ident[:],
                            )
                        gcols = slice(g0 * _P, (g0 + gn) * _P)
                        nc.vector.tensor_copy(xT[:, gcols], pt[:, : gn * _P])
                        nc.scalar.activation(xsq[:, gcols], pt[:, : gn * _P], SQ)
                        g0 += gn
                    for k in range(nblk):
                        kk = blk0 + k
                        bcols = slice(k * _P, (k + 1) * _P)
                        nc.tensor.matmul(
                            s_ps,
                            xT[:, bcols],
                            mask[:],
                            start=(kk == 0),
                            stop=(kk == _NBLK - 1),
                        )
                        nc.tensor.matmul(
                            q_ps,
                            xsq[:, bcols],
                            mask[:],
                            start=(kk == 0),
                            stop=(kk == _NBLK - 1),
                        )

                # res = 0.5*s^2 - 0.5*q
                s2h = spool.tile([_P, _E], f32, tag="s2h")
                res = spool.tile([_P, _E], f32, tag="res")
                nc.scalar.activation(s2h[:], s_ps, SQ, scale=HALF_SQRT)
                nc.vector.scalar_tensor_tensor(
                    res[:], q_ps, -0.5, s2h[:], op0=ALU.mult, op1=ALU.add
                )
                nc.sync.dma_start(out=out[rows, :], in_=res[:])
    nc.compile()
    return nc


def _consts():
    import ml_dtypes

    ident = np.eye(_P, dtype=ml_dtypes.bfloat16)
    mask = np.zeros((_P, _E), dtype=ml_dtypes.bfloat16)
    for j in range(_P):
        mask[j, j % _E] = 1
    return ident, mask


def _run(in_maps, **kwargs):
    from concourse.bass_utils import run_bass_kernel_spmd

    nc = _build()
    return run_bass_kernel_spmd(nc, in_maps, core_ids=list(range(_NCORES)), **kwargs)


def _shard(inputs: np.ndarray):
    x = np.ascontiguousarray(
        np.asarray(inputs, dtype=np.float32).reshape(_B, _ROW)
    )
    ident, mask = _consts()
    return [
        {
            "inputs": np.ascontiguousarray(x[i * _BS : (i + 1) * _BS]),
            "ident": ident,
            "mask": mask,
        }
        for i in range(_NCORES)
    ]


def kernel(
    inputs: np.ndarray,
    weight_attention: np.ndarray = None,
    weight_projection: np.ndarray = None,
    weight_bias: np.ndarray = None,
) -> np.ndarray:
    # weights are dead code (softmax over a size-1 axis == 1.0)
    res = _run(_shard(inputs))
    return np.concatenate([r["out"] for r in res.results], axis=0)
